# revision 1
# baseline (speedup 1.0000x reference)
"""Grouped (MoE-style) linear on 8 trn2 NeuronCores.

out[t] = hidden_states[t] @ weight[g(t)], where token t belongs to group g iff
offsets[g-1] <= t < offsets[g] (searchsorted right semantics; tokens at or past
offsets[-1] get zero output).

Strategy: expert-parallel. Core g owns weight[g] and the contiguous token run
of group g. Routing is done host-side (offsets are host data), each core runs
an identical Bass program: a [P_pad, 1024] x [1024, 1024] matmul tiled as
128-token blocks, contraction in 8 chunks of 128, PSUM-accumulated, fp32r
matmul (full PE rate; ~1.4e-4 relmax vs fp64 measured for this distribution).

Host packs per-core inputs so every DMA lands with >=4KB contiguous runs:
  xt[tb, p, k, tok] = X_g[tb*128 + tok, k*128 + p]   (transposed token block)
  w[p, k, n]        = W_g[k*128 + p, n]
"""
import numpy as np

import concourse.bass as bass
import concourse.tile as tile
from concourse import bacc, mybir
from concourse.bass_utils import run_bass_kernel_spmd

GROUPS = 8
TOKENS = 16384
IN_F = 1024
OUT_F = 1024
KCH = IN_F // 128  # contraction chunks




def build(ntb: int) -> bass.Bass:
    """One core's program: ntb 128-token blocks through a 1024x1024 expert."""
    f32 = mybir.dt.float32
    f32r = mybir.dt.float32r
    nc = bacc.Bacc()
    xt_d = nc.dram_tensor("xt", [ntb, 128, KCH, 128], f32r, kind="ExternalInput")
    w_d = nc.dram_tensor("w", [128, KCH, OUT_F], f32r, kind="ExternalInput")
    out_d = nc.dram_tensor("out", [ntb * 128, OUT_F], f32, kind="ExternalOutput")

    with tile.TileContext(nc) as tc:
        with (
            tc.tile_pool(name="wp", bufs=1) as wp,
            tc.tile_pool(name="xp", bufs=ntb) as xp,
            tc.tile_pool(name="op", bufs=4) as op,
            tc.tile_pool(name="ps", bufs=4, space="PSUM") as psp,
        ):
            wt = wp.tile([128, KCH, OUT_F], f32r)
            xts = []
            # startup: the first matmul needs only xt0's k=0 chunk (64KB) and
            # W chunk 0's first half (256KB); land those first, then the
            # next 3 token blocks (to keep 4 PSUM groups runnable while the
            # remaining W chunks stream in), then W k=1..7.
            xt0 = xp.tile([128, KCH, 128], f32r, tag="xt")
            nc.sync.dma_start(out=xt0[:, 0, :], in_=xt_d[0, :, 0, :])
            nc.scalar.dma_start(out=wt[:, 0, 0:512], in_=w_d[:, 0, 0:512])
            nc.sync.dma_start(out=xt0[:, 1:, :], in_=xt_d[0, :, 1:, :])
            nc.scalar.dma_start(out=wt[:, 0, 512:], in_=w_d[:, 0, 512:])
            xts.append(xt0)
            for t in range(1, min(4, ntb)):
                xtn = xp.tile([128, KCH, 128], f32r, tag="xt")
                nc.sync.dma_start(out=xtn[:], in_=xt_d[t])
                xts.append(xtn)
            for k in range(1, KCH):
                nc.sync.dma_start(out=wt[:, k, :], in_=w_d[:, k, :])
            for tb in range(ntb):
                if tb < len(xts):
                    xt = xts[tb]
                else:
                    xt = xp.tile([128, KCH, 128], f32r, tag="xt")
                    nc.sync.dma_start(out=xt[:], in_=xt_d[tb])
                ps = psp.tile([128, OUT_F], f32)
                if tb == 0:
                    # PE p-state warmup: re-issue the first matmul; start=True
                    # resets the bank each time so only the last one counts.
                    for _ in range(12):
                        nc.tensor.matmul(ps[:, 0:512], xt[:, 0, :],
                                         wt[:, 0, 0:512], start=True,
                                         stop=True, skip_group_check=True)
                for k in range(KCH):
                    for nb in range(OUT_F // 512):
                        nc.tensor.matmul(
                            ps[:, nb * 512:(nb + 1) * 512],
                            xt[:, k, :],
                            wt[:, k, nb * 512:(nb + 1) * 512],
                            start=(k == 0),
                            stop=(k == KCH - 1),
                        )
                ot = op.tile([128, OUT_F], f32)
                nc.scalar.copy(ot[:, 0:512], ps[:, 0:512])
                nc.vector.tensor_copy(ot[:, 512:1024], ps[:, 512:1024])
                if tb == ntb - 1:
                    nc.scalar.dma_start(out=out_d[tb * 128:(tb + 1) * 128, 0:512],
                                        in_=ot[:, 0:512])
                    nc.sync.dma_start(out=out_d[tb * 128:(tb + 1) * 128, 512:1024],
                                      in_=ot[:, 512:1024])
                else:
                    nc.scalar.dma_start(out=out_d[tb * 128:(tb + 1) * 128, :], in_=ot[:])
    nc.compile()
    return nc


def _pack_core(x_slice: np.ndarray, w_g: np.ndarray, ntb: int):
    n = x_slice.shape[0]
    xp = np.zeros((ntb * 128, IN_F), dtype=np.float32)
    xp[:n] = x_slice
    xt = np.ascontiguousarray(
        xp.reshape(ntb, 128, KCH, 128).transpose(0, 3, 2, 1)
    )
    wt = np.ascontiguousarray(w_g.reshape(KCH, 128, OUT_F).transpose(1, 0, 2))
    return xt, wt


def kernel(hidden_states: np.ndarray, weight: np.ndarray, offsets: np.ndarray,
           _trace: bool = False):
    hs = np.ascontiguousarray(hidden_states, dtype=np.float32)
    w = np.ascontiguousarray(weight, dtype=np.float32)
    off = np.asarray(offsets).astype(np.int64)

    ends = np.clip(off, 0, TOKENS)
    starts = np.concatenate(([0], ends[:-1]))
    starts = np.minimum(starts, ends)
    ns = ends - starts

    ntb = max(1, int(-(-ns.max() // 128)))
    nc = build(ntb)

    in_maps = []
    for g in range(GROUPS):
        xt, wt = _pack_core(hs[starts[g]:ends[g]], w[g], ntb)
        in_maps.append({"xt": xt, "w": wt})

    res = run_bass_kernel_spmd(nc, in_maps, list(range(GROUPS)), trace=_trace)

    out = np.zeros((TOKENS, OUT_F), dtype=np.float32)
    for g in range(GROUPS):
        if ns[g] > 0:
            out[starts[g]:ends[g]] = res.results[g]["out"][:ns[g]]
    if _trace:
        return out, res
    return out



# revision 4
# speedup vs baseline: 1.0398x; 1.0398x over previous
"""Grouped (MoE-style) linear on 8 trn2 NeuronCores.

out[t] = hidden_states[t] @ weight[g(t)], where token t belongs to group g iff
offsets[g-1] <= t < offsets[g] (searchsorted right semantics; tokens at or past
offsets[-1] get zero output).

Strategy: expert-parallel. Core g owns weight[g] and the contiguous token run
of group g. Routing is done host-side (offsets are host data), each core runs
an identical Bass program: a [P_pad, 1024] x [1024, 1024] matmul tiled as
128-token blocks, contraction in 8 chunks of 128, PSUM-accumulated.

Inputs are cast to bf16 on the host (PSUM accumulation stays fp32): the PE
runs bf16 at the same 1 col/cycle as fp32r, but input DMA halves (6.3MB/core),
which removes every DMA-gated stall: the critical first chunks (xt0 k=0 +
W k=0 cols 0:512) land ~0.6us after DMA spin-up, and the full W (2MB) lands
before tile0's contraction finishes. Measured rel-max error ~3e-3 for this
distribution (gate 2e-2).

PE p-state: the tensor engine clocks 1.2GHz until ~3us of continuous work,
then 2.4GHz. Dummy warmup matmuls on a memset SBUF tile (no DMA dependency)
ramp the clock during the ~8.5us DMA-startup shadow, costing zero useful time.

Host packs per-core inputs so every DMA lands with big contiguous runs:
  xt[tb, p, k, tok] = X_g[tb*128 + tok, k*128 + p]   (transposed token block)
  w[p, k, n]        = W_g[k*128 + p, n]
"""
import ml_dtypes
import numpy as np

import concourse.bass as bass
import concourse.tile as tile
from concourse import bacc, mybir
from concourse.bass_utils import run_bass_kernel_spmd

GROUPS = 8
TOKENS = 16384
IN_F = 1024
OUT_F = 1024
KCH = IN_F // 128  # contraction chunks
NWARM = 12         # dummy ramp matmuls of 256 cols each


def build(ntb: int) -> bass.Bass:
    """One core's program: ntb 128-token blocks through a 1024x1024 expert."""
    f32 = mybir.dt.float32
    bf16 = mybir.dt.bfloat16
    nc = bacc.Bacc()
    xt_d = nc.dram_tensor("xt", [ntb, 128, KCH, 128], bf16, kind="ExternalInput")
    w_d = nc.dram_tensor("w", [128, KCH, OUT_F], bf16, kind="ExternalInput")
    out_d = nc.dram_tensor("out", [ntb * 128, OUT_F], f32, kind="ExternalOutput")

    with tile.TileContext(nc) as tc:
        with (
            tc.tile_pool(name="wp", bufs=1) as wp,
            tc.tile_pool(name="xp", bufs=ntb) as xp,
            tc.tile_pool(name="op", bufs=4) as op,
            tc.tile_pool(name="fp", bufs=1) as fp,
            tc.tile_pool(name="ps", bufs=4, space="PSUM") as psp,
        ):
            # PE p-state ramp: memset a dummy tile (no DMA dep) and chain
            # matmuls on it so the clock reaches 2.4GHz inside the DMA
            # startup shadow. Results land in the first psum tile and are
            # discarded (first real matmul has start=True).
            dummy = fp.tile([128, 512], bf16, tag="warm")
            nc.gpsimd.memset(dummy[:], 0)

            wt = wp.tile([128, KCH, OUT_F], bf16)
            xts = []
            # Critical first chunks, one per input queue: xt0's k=0 chunk
            # (32KB) on sync, W chunk 0 cols 0:512 (128KB) on scalar.
            xt0 = xp.tile([128, KCH, 128], bf16, tag="xt")
            nc.sync.dma_start(out=xt0[:, 0, :], in_=xt_d[0, :, 0, :])
            nc.scalar.dma_start(out=wt[:, 0, 0:512], in_=w_d[:, 0, 0:512])
            nc.sync.dma_start(out=xt0[:, 1:, :], in_=xt_d[0, :, 1:, :])
            xts.append(xt0)
            # W is needed in full by the end of tile0's contraction; split it
            # across both queues so it lands by ~14us.
            for k in range(KCH):
                nc.scalar.dma_start(out=wt[:, k, 0:512] if k else wt[:, k, 512:1024],
                                    in_=w_d[:, k, 0:512] if k else w_d[:, k, 512:1024])
            for k in range(1, KCH):
                nc.sync.dma_start(out=wt[:, k, 512:1024], in_=w_d[:, k, 512:1024])
            for t in range(1, ntb):
                xtn = xp.tile([128, KCH, 128], bf16, tag="xt")
                nc.sync.dma_start(out=xtn[:], in_=xt_d[t])
                xts.append(xtn)
            ps0 = psp.tile([128, OUT_F], f32, tag="acc")
            for _ in range(NWARM):
                nc.tensor.matmul(ps0[:, 0:256], dummy[:, 0:128], dummy[:, 0:256],
                                 start=True, stop=True, skip_group_check=True)
            for tb in range(ntb):
                xt = xts[tb]
                ps = ps0 if tb == 0 else psp.tile([128, OUT_F], f32, tag="acc")
                for k in range(KCH):
                    for nb in range(OUT_F // 512):
                        nc.tensor.matmul(
                            ps[:, nb * 512:(nb + 1) * 512],
                            xt[:, k, :],
                            wt[:, k, nb * 512:(nb + 1) * 512],
                            start=(k == 0),
                            stop=(k == KCH - 1),
                        )
                if tb == ntb - 1:
                    # Tail flush: dedicated tiles (no pool anti-deps), copies
                    # run in parallel on scalar+vector, halves DMA out on two
                    # otherwise-idle queues.
                    ota = fp.tile([128, 512], f32, tag="ota")
                    otb = fp.tile([128, 512], f32, tag="otb")
                    nc.scalar.copy(ota[:], ps[:, 0:512])
                    nc.vector.tensor_copy(otb[:], ps[:, 512:1024])
                    nc.sync.dma_start(out=out_d[tb * 128:(tb + 1) * 128, 0:512],
                                      in_=ota[:])
                    nc.scalar.dma_start(out=out_d[tb * 128:(tb + 1) * 128, 512:1024],
                                        in_=otb[:])
                else:
                    ot = op.tile([128, OUT_F], f32)
                    nc.scalar.copy(ot[:, 0:512], ps[:, 0:512])
                    nc.vector.tensor_copy(ot[:, 512:1024], ps[:, 512:1024])
                    nc.scalar.dma_start(out=out_d[tb * 128:(tb + 1) * 128, :], in_=ot[:])
    nc.compile()
    return nc


def _pack_core(x_slice: np.ndarray, w_g: np.ndarray, ntb: int):
    n = x_slice.shape[0]
    xp = np.zeros((ntb * 128, IN_F), dtype=np.float32)
    xp[:n] = x_slice
    xt = np.ascontiguousarray(
        xp.reshape(ntb, 128, KCH, 128).transpose(0, 3, 2, 1).astype(ml_dtypes.bfloat16)
    )
    wt = np.ascontiguousarray(
        w_g.reshape(KCH, 128, OUT_F).transpose(1, 0, 2).astype(ml_dtypes.bfloat16)
    )
    return xt, wt


def kernel(hidden_states: np.ndarray, weight: np.ndarray, offsets: np.ndarray,
           _trace: bool = False):
    hs = np.ascontiguousarray(hidden_states, dtype=np.float32)
    w = np.ascontiguousarray(weight, dtype=np.float32)
    off = np.asarray(offsets).astype(np.int64)

    ends = np.clip(off, 0, TOKENS)
    starts = np.concatenate(([0], ends[:-1]))
    starts = np.minimum(starts, ends)
    ns = ends - starts

    ntb = max(1, int(-(-ns.max() // 128)))
    nc = build(ntb)

    in_maps = []
    for g in range(GROUPS):
        xt, wt = _pack_core(hs[starts[g]:ends[g]], w[g], ntb)
        in_maps.append({"xt": xt, "w": wt})

    res = run_bass_kernel_spmd(nc, in_maps, list(range(GROUPS)), trace=_trace)

    out = np.zeros((TOKENS, OUT_F), dtype=np.float32)
    for g in range(GROUPS):
        if ns[g] > 0:
            out[starts[g]:ends[g]] = res.results[g]["out"][:ns[g]]
    if _trace:
        return out, res
    return out


# revision 6
# speedup vs baseline: 1.0624x; 1.0218x over previous
"""Grouped (MoE-style) linear on 8 trn2 NeuronCores.

out[t] = hidden_states[t] @ weight[g(t)], where token t belongs to group g iff
offsets[g-1] <= t < offsets[g] (searchsorted right semantics; tokens at or past
offsets[-1] get zero output).

Strategy: expert-parallel. Core g owns weight[g] and the contiguous token run
of group g. Routing is done host-side (offsets are host data), each core runs
an identical Bass program: a [P_pad, 1024] x [1024, 1024] matmul tiled as
128-token blocks, contraction in 8 chunks of 128, PSUM-accumulated.

Inputs are cast to bf16 on the host (PSUM accumulation stays fp32): same
1 col/cycle PE rate as fp32r but half the input DMA. Measured rel-max error
~2.3e-3 for this distribution (gate 2e-2).

Schedule, driven by trace analysis:
- PE p-state: the tensor engine clocks 1.2GHz until ~3us of continuous work,
  then 2.4GHz. Dummy warmup matmuls on a memset SBUF tile (no DMA dep) ramp
  the clock inside the ~8.5us DMA-startup shadow at zero useful-time cost.
- W (2MB) streams in at ~150KB/us/queue and only fully lands ~15-20us in; a
  tile-major schedule starves on it. Phase 1 runs the first 4 token blocks
  k-OUTER (all blocks consume W chunk k, then k+1...) so compute tracks W's
  arrival order. Phase 2 (remaining blocks) is tile-major with all W resident.
- PSUM is 8 x [128,512] half-tiles (one bank each): scalar copies the nb=0
  half while vector copies nb=1 in parallel (a shared [128,1024] tensor gets
  serialized by Tile's bank tracker), and the last block runs nb-outer so its
  first half is copied+DMA'd out under the shadow of its second half.

Host packs per-core inputs so every DMA lands with big contiguous runs:
  xt[tb, p, k, tok] = X_g[tb*128 + tok, k*128 + p]   (transposed token block)
  w[p, k, n]        = W_g[k*128 + p, n]
"""
import ml_dtypes
import numpy as np

import concourse.bass as bass
import concourse.tile as tile
from concourse import bacc, mybir
from concourse.bass_utils import run_bass_kernel_spmd

GROUPS = 8
TOKENS = 16384
IN_F = 1024
OUT_F = 1024
KCH = IN_F // 128  # contraction chunks
NWARM = 8          # dummy ramp matmuls of 256 cols each
PH1 = 4            # token blocks in the k-outer phase


def build(ntb: int) -> bass.Bass:
    """One core's program: ntb 128-token blocks through a 1024x1024 expert."""
    f32 = mybir.dt.float32
    bf16 = mybir.dt.bfloat16
    nc = bacc.Bacc()
    xt_d = nc.dram_tensor("xt", [ntb, 128, KCH, 128], bf16, kind="ExternalInput")
    w_d = nc.dram_tensor("w", [128, KCH, OUT_F], bf16, kind="ExternalInput")
    out_d = nc.dram_tensor("out", [ntb * 128, OUT_F], f32, kind="ExternalOutput")

    p1 = min(PH1, ntb)

    with tile.TileContext(nc) as tc:
        with (
            tc.tile_pool(name="wp", bufs=1) as wp,
            tc.tile_pool(name="xp", bufs=ntb) as xp,
            tc.tile_pool(name="op", bufs=4) as op,
            tc.tile_pool(name="fp", bufs=1) as fp,
            tc.tile_pool(name="ps", bufs=8, space="PSUM") as psp,
        ):
            # PE p-state ramp tile (no DMA dependency).
            dummy = fp.tile([128, 256], bf16, tag="warm")
            nc.gpsimd.memset(dummy[:], 0)

            wt = wp.tile([128, KCH, OUT_F], bf16)
            xts = []
            # Critical first chunks, one per input queue: xt0's k=0 chunk on
            # sync, W chunk 0 cols 0:512 on scalar. Then W halves split across
            # both queues in k order, interleaved with the phase-1 xt tiles so
            # W chunk k always lands before its phase-1 round.
            xt0 = xp.tile([128, KCH, 128], bf16, tag="xt")
            nc.sync.dma_start(out=xt0[:, 0, :], in_=xt_d[0, :, 0, :])
            nc.scalar.dma_start(out=wt[:, 0, 0:512], in_=w_d[:, 0, 0:512])
            nc.sync.dma_start(out=wt[:, 0, 512:1024], in_=w_d[:, 0, 512:1024])
            nc.sync.dma_start(out=xt0[:, 1:, :], in_=xt_d[0, :, 1:, :])
            xts.append(xt0)
            for k in range(1, KCH):
                nc.scalar.dma_start(out=wt[:, k, 0:512], in_=w_d[:, k, 0:512])
            for k in range(1, KCH):
                nc.sync.dma_start(out=wt[:, k, 512:1024], in_=w_d[:, k, 512:1024])
                if k < p1:
                    xtn = xp.tile([128, KCH, 128], bf16, tag="xt")
                    nc.sync.dma_start(out=xtn[:], in_=xt_d[k])
                    xts.append(xtn)
            for t in range(p1, ntb):
                xtn = xp.tile([128, KCH, 128], bf16, tag="xt")
                nc.sync.dma_start(out=xtn[:], in_=xt_d[t])
                xts.append(xtn)

            # Half-width PSUM accumulators: pa[tb] = cols 0:512, pb = 512:1024.
            pa = {}
            pb = {}
            for tb in range(p1):
                pa[tb] = psp.tile([128, 512], f32, tag="acc", name=f"pa{tb}")
                pb[tb] = psp.tile([128, 512], f32, tag="acc", name=f"pb{tb}")

            for _ in range(NWARM):
                nc.tensor.matmul(pa[0][:, 0:256], dummy[:, 0:128], dummy[:],
                                 start=True, stop=True, skip_group_check=True)

            def flush(tb, a, b):
                if tb == ntb - 1:
                    ota = fp.tile([128, 512], f32, tag="ota")
                    otb = fp.tile([128, 512], f32, tag="otb")
                    nc.scalar.copy(ota[:], a[:])
                    nc.sync.dma_start(out=out_d[tb * 128:(tb + 1) * 128, 0:512],
                                      in_=ota[:])
                    nc.vector.tensor_copy(otb[:], b[:])
                    nc.scalar.dma_start(out=out_d[tb * 128:(tb + 1) * 128, 512:1024],
                                        in_=otb[:])
                else:
                    ot = op.tile([128, OUT_F], f32)
                    nc.scalar.copy(ot[:, 0:512], a[:])
                    nc.vector.tensor_copy(ot[:, 512:1024], b[:])
                    nc.scalar.dma_start(out=out_d[tb * 128:(tb + 1) * 128, :], in_=ot[:])

            # Phase 1: k-outer over the first p1 blocks.
            for k in range(KCH):
                for tb in range(p1):
                    nc.tensor.matmul(pa[tb][:], xts[tb][:, k, :], wt[:, k, 0:512],
                                     start=(k == 0), stop=(k == KCH - 1))
                    nc.tensor.matmul(pb[tb][:], xts[tb][:, k, :], wt[:, k, 512:1024],
                                     start=(k == 0), stop=(k == KCH - 1))
            for tb in range(p1):
                flush(tb, pa[tb], pb[tb])

            # Phase 2: tile-major, W fully resident.
            for tb in range(p1, ntb):
                a = psp.tile([128, 512], f32, tag="acc")
                b = psp.tile([128, 512], f32, tag="acc")
                if tb == ntb - 1:
                    # nb-outer: finish cols 0:512 first so their copy+DMA
                    # overlap the cols 512:1024 matmuls.
                    for k in range(KCH):
                        nc.tensor.matmul(a[:], xts[tb][:, k, :], wt[:, k, 0:512],
                                         start=(k == 0), stop=(k == KCH - 1))
                    for k in range(KCH):
                        nc.tensor.matmul(b[:], xts[tb][:, k, :], wt[:, k, 512:1024],
                                         start=(k == 0), stop=(k == KCH - 1))
                else:
                    for k in range(KCH):
                        nc.tensor.matmul(a[:], xts[tb][:, k, :], wt[:, k, 0:512],
                                         start=(k == 0), stop=(k == KCH - 1))
                        nc.tensor.matmul(b[:], xts[tb][:, k, :], wt[:, k, 512:1024],
                                         start=(k == 0), stop=(k == KCH - 1))
                flush(tb, a, b)
    nc.compile()
    return nc


def _pack_core(x_slice: np.ndarray, w_g: np.ndarray, ntb: int):
    n = x_slice.shape[0]
    xp = np.zeros((ntb * 128, IN_F), dtype=np.float32)
    xp[:n] = x_slice
    xt = np.ascontiguousarray(
        xp.reshape(ntb, 128, KCH, 128).transpose(0, 3, 2, 1).astype(ml_dtypes.bfloat16)
    )
    wt = np.ascontiguousarray(
        w_g.reshape(KCH, 128, OUT_F).transpose(1, 0, 2).astype(ml_dtypes.bfloat16)
    )
    return xt, wt


def kernel(hidden_states: np.ndarray, weight: np.ndarray, offsets: np.ndarray,
           _trace: bool = False):
    hs = np.ascontiguousarray(hidden_states, dtype=np.float32)
    w = np.ascontiguousarray(weight, dtype=np.float32)
    off = np.asarray(offsets).astype(np.int64)

    ends = np.clip(off, 0, TOKENS)
    starts = np.concatenate(([0], ends[:-1]))
    starts = np.minimum(starts, ends)
    ns = ends - starts

    ntb = max(1, int(-(-ns.max() // 128)))
    nc = build(ntb)

    in_maps = []
    for g in range(GROUPS):
        xt, wt = _pack_core(hs[starts[g]:ends[g]], w[g], ntb)
        in_maps.append({"xt": xt, "w": wt})

    res = run_bass_kernel_spmd(nc, in_maps, list(range(GROUPS)), trace=_trace)

    out = np.zeros((TOKENS, OUT_F), dtype=np.float32)
    for g in range(GROUPS):
        if ns[g] > 0:
            out[starts[g]:ends[g]] = res.results[g]["out"][:ns[g]]
    if _trace:
        return out, res
    return out
